# revision 9
# baseline (speedup 1.0000x reference)
"""ACE loss kernel for TRN2, data-parallel over 8 NeuronCores.

Math (per sample b, with targets y[b, 0:8] and logits x[b, c, t]):
  m[b,t]   = max_c x[b,c,t]
  cnt[b,j] = #{t : x[b, y[b,j], t] == m[b,t]}        == n_k[b, y[b,j]] (no ties)
  dup[b,j] = multiplicity of y[b,j] within y[b,:]    == y_k[b, y[b,j]]
Only target classes contribute to the masked loss, so the full 128-bin
argmax histogram is never materialized:
  n_sum[b] = sum_j cnt/dup   (each distinct class counted once)
  n_p[b,j] = max(cnt / max(n_sum,1), EPS)
  loss[b]  = sum_j n_p * (-log(dup/8)) / dup
  out      = mean_b loss

v4 design (f16 stream, DVE/ACT split):
  - Host casts x to f16: by monotonicity of round-to-nearest,
    max(f16 a, f16 b) == f16(max(a,b)), so this is bit-identical to the
    old f32 kernel's first-level rounding while HALVING the HBM stream
    (32 -> 16.8 MiB/core).  Measured stream rate ~430 GB/s/core.
  - All tree levels are f16 TT-max at DVE 2x_1P (2 out/cycle; the only
    DVE op family with an accelerated mode that can reduce).  Measured
    v2/v3 DVE busy matched the cycle model within 1%.
  - Tiles 0,1 get per-tile trees (work available immediately -> no DVE
    idle during the pipeline ramp); tiles (2,3) and (4,5) get 3D-AP
    pair-batched levels (one op per level for both tiles, op overhead
    halved) since DVE backlog exists by then; tile 6 whole; tile 7 in
    quarters+eighths with a running 512-wide fold so only ~2 us of tree
    work follows the last byte.
  - Counts for tiles 0-5 run on the otherwise-idle ACT engine:
    DVE computes d = m - xg (exact f16 sub), ACT accumulates
    exp(-16384*d) per (tile,j) row -- matches contribute exp(0)=1,
    non-matches (d >= ~half-ulp) < 3e-4.  48 small ACT ops finish
    mid-stream.  Tiles 6,7 count on DVE (is_equal + reduce) to avoid
    queueing behind ACT at the tail (v3 lost 7 us there).
  - GpSimd cannot help: walrus codegen rejects TT ops on Pool, and
    free-axis tensor_reduce is Vector-only.  pool/reduce are 1x-mode.
  - One batched epilogue, PE f32 dot with ones collapses partitions,
    scalar leaves through a 4-byte single-descriptor DMA.
Each core returns one f32; the host sums 8 of them and divides by B.
"""

import numpy as np

B, C, T, L = 8192, 128, 64, 8
N_CORES = 8
B_SH = B // N_CORES          # 1024 samples per core
NT = B_SH // 128             # 8 tiles of 128 samples
EPS = 1e-5

_CACHE = {}


def _build_nc():
    import sys
    if "/opt/trn_rl_repo" not in sys.path:
        sys.path.insert(0, "/opt/trn_rl_repo")
    from concourse import bacc, mybir
    from concourse.tile import TileContext

    f32 = mybir.dt.float32
    f16 = mybir.dt.float16
    AX = mybir.AxisListType
    OP = mybir.AluOpType

    CT = C * T            # 8192 elems per sample
    Q = CT // 4           # 2048
    E = CT // 8           # 1024
    H = CT // 2           # 4096

    nc = bacc.Bacc("TRN2")
    x = nc.declare_dram_parameter("x", [B_SH, CT], f16, isOutput=False)
    XGO = NT * L          # xg columns start here inside yga
    yg = nc.declare_dram_parameter(
        "yg", [128, XGO + NT * L * T], f16, isOutput=False
    )
    out = nc.declare_dram_parameter("out", [1, 1], f32, isOutput=True)

    with TileContext(nc) as tc:
        with (
            tc.tile_pool(name="xp", bufs=2) as xp,
            tc.tile_pool(name="hp", bufs=2) as hp,
            tc.tile_pool(name="sp", bufs=3) as sp,
            tc.tile_pool(name="cp", bufs=1) as cp,
            tc.tile_pool(name="ps", bufs=1, space="PSUM") as pp,
        ):
            # ---- whole-run tiles ----
            yga = cp.tile([128, XGO + NT * L * T], f16)
            mh = cp.tile([128, NT * T], f16)      # per-tile class-max rows
            cnta = cp.tile([128, NT * L], f32)
            ones = cp.tile([128, 1], f32)
            nc.gpsimd.memset(ones[:, :], 1.0)

            ycf = cp.tile([128, NT * L], f32)
            eq8 = cp.tile([128, NT * L * L], f32)
            dup = cp.tile([128, NT * L], f32)
            rd = cp.tile([128, NT * L], f32)
            lg = cp.tile([128, NT * L], f32)
            wgt = cp.tile([128, NT * L], f32)

            def ymath():
                # dup[b,j] = multiplicity of y_j in y[b,:]
                nc.vector.tensor_copy(out=ycf[:, :], in_=yga[:, 0:XGO])
                nc.vector.tensor_tensor(
                    out=eq8[:, :].rearrange("p (k a b) -> p k a b", a=L, b=L),
                    in0=ycf[:, :].rearrange("p (k a) -> p k a", a=L)
                    .unsqueeze(3).to_broadcast([128, NT, L, L]),
                    in1=ycf[:, :].rearrange("p (k a) -> p k a", a=L)
                    .unsqueeze(2).to_broadcast([128, NT, L, L]),
                    op=OP.is_equal,
                )
                nc.vector.reduce_sum(
                    out=dup[:, :],
                    in_=eq8[:, :].rearrange("p (k a b) -> p k a b", a=L, b=L),
                    axis=AX.X,
                )
                nc.vector.reciprocal(out=rd[:, :], in_=dup[:, :])
                nc.scalar.activation(
                    out=lg[:, :], in_=dup[:, :],
                    func=mybir.ActivationFunctionType.Ln, scale=1.0 / L,
                )
                # wgt = -log(dup/8)/dup
                nc.vector.scalar_tensor_tensor(
                    out=wgt[:, :], in0=lg[:, :], scalar=-1.0,
                    in1=rd[:, :], op0=OP.mult, op1=OP.mult,
                )

            # f16 pair-max: xt[lo:lo+n] -> ht[hoff:hoff+n/2]; pairs
            # (lo+i, lo+n/2+i) are 64-aligned so t-columns stay aligned
            def l1(xt, lo, n, ht, hoff):
                nc.vector.tensor_tensor(
                    out=ht[:, hoff:hoff + n // 2],
                    in0=xt[:, lo:lo + n // 2],
                    in1=xt[:, lo + n // 2:lo + n],
                    op=OP.max,
                )

            # fp16 in-place max tree over t[:, lo:lo+n] down to W wide
            def tree16(t, lo, n, W=T, last_out=None):
                w = n
                while w > W:
                    h = w // 2
                    dst = (last_out if (last_out is not None and h == W)
                           else t[:, lo:lo + h])
                    nc.vector.tensor_tensor(
                        out=dst, in0=t[:, lo:lo + h],
                        in1=t[:, lo + h:lo + w], op=OP.max,
                    )
                    w = h

            # batched tree over a PAIR of tiles in ht[:, 0:2H] as [p,2,H]
            def tree_pair(ht, mcol2):
                view = ht[:, 0:2 * H].rearrange("p (g n) -> p g n", g=2)
                w = H
                while w > T:
                    h = w // 2
                    dst = mcol2 if h == T else view[:, :, 0:h]
                    nc.vector.tensor_tensor(
                        out=dst, in0=view[:, :, 0:h],
                        in1=view[:, :, h:w], op=OP.max,
                    )
                    w = h

            # DVE computes d = m - xg; ACT turns each (k,j) row into a
            # count via exp(-16384*d) with accum_out (tiles 0-5)
            def count_act(k0, g):
                d = sp.tile([128, 2 * L * T], f16, tag="d")
                e = sp.tile([128, 2 * L * T], f16, tag="e")
                nc.vector.tensor_tensor(
                    out=d[:, 0:g * L * T].rearrange(
                        "p (g l t) -> p g l t", g=g, l=L),
                    in0=mh[:, k0 * T:(k0 + g) * T].rearrange(
                        "p (g t) -> p g t", g=g
                    ).unsqueeze(2).to_broadcast([128, g, L, T]),
                    in1=yga[:, XGO + k0 * L * T:XGO + (k0 + g) * L * T].rearrange(
                        "p (g l t) -> p g l t", g=g, l=L
                    ),
                    op=OP.subtract,
                )
                for s in range(g * L):
                    slot = k0 * L + s
                    nc.scalar.activation(
                        out=e[:, s * T:(s + 1) * T],
                        in_=d[:, s * T:(s + 1) * T],
                        func=mybir.ActivationFunctionType.Exp,
                        scale=-16384.0,
                        accum_out=cnta[:, slot:slot + 1],
                    )

            # DVE-only count (tiles 6,7: no ACT queue at the tail)
            def count_dve(k):
                eq = sp.tile([128, 2 * L * T], f16, tag="d")
                nc.vector.tensor_tensor(
                    out=eq[:, 0:L * T].rearrange("p (l t) -> p l t", l=L),
                    in0=yga[:, XGO + k * L * T:XGO + (k + 1) * L * T].rearrange(
                        "p (l t) -> p l t", l=L
                    ),
                    in1=mh[:, k * T:(k + 1) * T].unsqueeze(1)
                    .to_broadcast([128, L, T]),
                    op=OP.is_equal,
                )
                nc.vector.reduce_sum(
                    out=cnta[:, k * L:(k + 1) * L],
                    in_=eq[:, 0:L * T].rearrange("p (l t) -> p l t", l=L),
                    axis=AX.X,
                )

            # NOTE: a compute op reading a pool tile waits for ALL DMA
            # writers of that tile (deps collapse to tile granularity),
            # so chunked streaming only pipelines when each chunk lives
            # in its OWN pool tile.  Measured: v2's quarter-l1 started
            # exactly when the 4th quarter landed.

            # ---- tile 0: quarters in separate tiles (earliest DVE start) ----
            ht = hp.tile([128, H], f16, tag="hts")
            for c in range(4):
                xq = xp.tile([128, Q], f16, tag="xq", bufs=4)
                nc.sync.dma_start(
                    out=xq[:, :], in_=x[0:128, c * Q:(c + 1) * Q]
                )
                l1(xq, 0, Q, ht, c * (Q // 2))
            # yg rides the ring after tile 0: lands before tile-0's
            # count needs xg; ymath fills an early DVE gap
            nc.sync.dma_start(out=yga[:, :], in_=yg[:, :])
            tree16(ht, 0, H, T, last_out=mh[:, 0:T])
            ymath()
            count_act(0, 1)

            # ---- tile 1: halves in separate tiles ----
            ht = hp.tile([128, H], f16, tag="hts")
            for c in range(2):
                xh = xp.tile([128, H], f16, tag="xh", bufs=2)
                nc.sync.dma_start(
                    out=xh[:, :], in_=x[128:256, c * H:(c + 1) * H]
                )
                l1(xh, 0, H, ht, c * Q)
            tree16(ht, 0, H, T, last_out=mh[:, T:2 * T])
            count_act(1, 1)

            # ---- pair groups: tiles (2,3), (4,5) ----
            for g in range(2):
                k0 = 2 + 2 * g
                xt = xp.tile([128, 2 * CT], f16, tag="xt")
                ht = hp.tile([128, 2 * H], f16, tag="ht")
                for i in range(2):
                    k = k0 + i
                    row = slice(k * 128, (k + 1) * 128)
                    nc.sync.dma_start(
                        out=xt[:, i * CT:(i + 1) * CT], in_=x[row, :]
                    )
                    l1(xt, i * CT, CT, ht, i * H)
                tree_pair(ht, mh[:, k0 * T:(k0 + 2) * T].rearrange(
                    "p (g t) -> p g t", g=2))
                count_act(k0, 2)

            # ---- tile 6: whole-tile load, full tree ----
            xt = xp.tile([128, 2 * CT], f16, tag="xt")
            ht = hp.tile([128, H], f16, tag="hts")
            row = slice(6 * 128, 7 * 128)
            nc.sync.dma_start(out=xt[:, 0:CT], in_=x[row, :])
            l1(xt, 0, CT, ht, 0)
            tree16(ht, 0, H, T, last_out=mh[:, 6 * T:7 * T])
            count_act(6, 1)

            # ---- tile 7: 2 quarters + 4 eighths in separate tiles,
            # running fold at 512 -> only ~1.5us of tree follows last byte
            ht = hp.tile([128, H], f16, tag="hts")
            row = slice(7 * 128, 8 * 128)
            pieces = ([(0, Q), (Q, Q)] + [(H + c * E, E) for c in range(4)])
            hoff = 0
            for pi, (lo, n) in enumerate(pieces):
                xq = xp.tile([128, Q], f16, tag="xq", bufs=4)
                nc.sync.dma_start(out=xq[:, 0:n], in_=x[row, lo:lo + n])
                l1(xq, 0, n, ht, hoff)
                tree16(ht, hoff, n // 2, 512)
                if pi > 0:
                    nc.vector.tensor_tensor(
                        out=ht[:, 0:512], in0=ht[:, 0:512],
                        in1=ht[:, hoff:hoff + 512], op=OP.max,
                    )
                hoff += n // 2
            tree16(ht, 0, 512, T, last_out=mh[:, 7 * T:8 * T])
            count_dve(7)

            # ---- batched epilogue over all 8 tile columns ----
            nd = cp.tile([128, NT * L], f32)
            nsum = cp.tile([128, NT], f32)
            inv = cp.tile([128, NT], f32)
            npj = cp.tile([128, NT * L], f32)
            lj = cp.tile([128, NT * L], f32)
            nc.vector.tensor_mul(out=nd[:, :], in0=cnta[:, :], in1=rd[:, :])
            nc.vector.reduce_sum(
                out=nsum[:, :],
                in_=nd[:, :].rearrange("p (k j) -> p k j", j=L),
                axis=AX.X,
            )
            nc.vector.tensor_scalar_max(out=nsum[:, :], in0=nsum[:, :], scalar1=1.0)
            nc.vector.reciprocal(out=inv[:, :], in_=nsum[:, :])
            nc.vector.tensor_tensor(
                out=npj[:, :].rearrange("p (k j) -> p k j", j=L),
                in0=cnta[:, :].rearrange("p (k j) -> p k j", j=L),
                in1=inv[:, :].unsqueeze(2).to_broadcast([128, NT, L]),
                op=OP.mult,
            )
            nc.vector.tensor_scalar_max(out=npj[:, :], in0=npj[:, :], scalar1=EPS)
            nc.vector.tensor_mul(out=lj[:, :], in0=npj[:, :], in1=wgt[:, :])
            acc = cp.tile([128, 1], f32)
            nc.vector.reduce_sum(
                out=acc[:, :],
                in_=lj[:, :].rearrange("p (k j) -> p k j", j=L),
                axis=AX.XY,
            )
            # collapse partitions: PE f32 dot with ones -> PSUM [1,1] -> SBUF
            psc = pp.tile([1, 1], f32)
            nc.tensor.matmul(psc[:, :], acc[:, :], ones[:, :],
                             start=True, stop=True)
            outv = cp.tile([1, 1], f32)
            nc.vector.tensor_copy(out=outv[:, :], in_=psc[:, :])
            nc.sync.dma_start(out=out[:, :], in_=outv[:, :])
    nc.compile()
    return nc


def _shard_inputs(x, y, target_lengths):
    """Numpy-side sharding, f16 cast, target-row pre-gather, layouts."""
    x = np.asarray(x, dtype=np.float32)
    y = np.asarray(y, dtype=np.int32)
    y2 = y.reshape(B, L)  # target_lengths is L for every sample (spec'd)
    x3 = x.reshape(B, C, T)
    # f16 stream: max(f16 a, f16 b) == f16(max(a, b)) (monotone rounding),
    # so device results are identical to computing f16 maxes on f32 input
    x16 = x3.astype(np.float16)
    xg_all = np.take_along_axis(
        x16, y2[:, :, None].astype(np.int64), axis=1
    )

    in_maps = []
    for i in range(N_CORES):
        sl = slice(i * B_SH, (i + 1) * B_SH)
        xs = np.ascontiguousarray(x16[sl].reshape(B_SH, C * T))
        xgs = xg_all[sl].reshape(NT, 128, L * T).transpose(1, 0, 2).reshape(128, -1)
        # classes 0..127 are exact in fp16; pack [yc | xg] as one buffer
        ycs = (y2[sl].reshape(NT, 128, L).transpose(1, 0, 2)
               .reshape(128, -1).astype(np.float16))
        ygs = np.ascontiguousarray(np.concatenate([ycs, xgs], axis=1))
        in_maps.append({"x": xs, "yg": ygs})
    return in_maps


def kernel(x, y, target_lengths):
    import sys
    if "/opt/trn_rl_repo" not in sys.path:
        sys.path.insert(0, "/opt/trn_rl_repo")
    from concourse.bass_utils import run_bass_kernel_spmd

    if "nc" not in _CACHE:
        _CACHE["nc"] = _build_nc()
    nc = _CACHE["nc"]

    in_maps = _shard_inputs(x, y, target_lengths)
    res = run_bass_kernel_spmd(nc, in_maps, core_ids=list(range(N_CORES)))
    total = np.float64(0.0)
    for r in res.results:
        total += np.float64(np.asarray(r["out"]).reshape(()))
    return np.float32(total / B)
